# revision 14
# baseline (speedup 1.0000x reference)
"""Local (windowed causal) attention with RoPE — Trainium2 Bass kernel.

Problem: B=4, H=16, T=4096, E=64, WINDOW=128, look_backward=1, causal.
Sharding: merged batch*heads (64 rows) split 8 per NeuronCore across 8 cores.

v3 design (all matmul operands fp16, fp32 accumulation):
- q/k/v loaded per bh via SWDGE (gpsimd) with f32->bf16 cast, "(c)-layout"
  [128, nw*64] (tile[p, w*64+e] = x[w*128+p, e]).
- RoPE in bf16 on DVE (whole-bh ops, 2x mode), host-precomputed bf16 tables.
- PE transposes into one merged PSUM bank [64, 1024] per 4 windows
  (q cols 0:512, k cols 512:1024) -> qT/kT [64, T] bf16; copies on ACT.
- Scores: 4 windows per PSUM tile [128, 1024] f32; st[j,i] via
  matmul(lhsT=kT_w [64,128], rhs=qT[w..w+2) [64,256]); one ACT exp
  (scale=8^-0.5... E**-0.5) -> E bf16 [128,1024]; DVE causal mask on cur
  halves (strided, mul by 0/1 bf16).
- PV: matmul(lhsT=E block [128,128] bf16, rhs=[v|1] bf16 [128,65]) accumulating
  out + softmax denominator in PSUM [128, 260] (4 windows); batched DVE
  reciprocal + broadcast-mul normalize into a per-bh out buffer; one 1MB
  store per bh on the sync ring.
"""

from contextlib import ExitStack

import numpy as np

import concourse.bass as bass
import concourse.bacc as bacc
import concourse.mybir as mybir
from concourse import tile
from concourse import bass_utils

F32 = mybir.dt.float32
F16 = mybir.dt.float16
NP_F16 = np.float16
E = 64
W = 128
HALF = 32
B, H, T = 4, 16, 4096
N_CORES = 8
N_BH = (B * H) // N_CORES
NW = T // W
WT = 4  # windows per transpose/score/output batch


# ---------------------------------------------------------------- host consts
def _rope_tables(t_len=T):
    nw = t_len // W
    inv_freq = 1.0 / (10000.0 ** (np.arange(0, E, 2, dtype=np.float32) / E))
    t = np.arange(t_len, dtype=np.float32)
    freqs = np.outer(t, inv_freq)
    emb = np.concatenate([freqs, freqs], axis=-1)
    cos = np.cos(emb).astype(np.float32)
    sin = np.sin(emb).astype(np.float32)
    sinA = np.concatenate([-sin[:, :HALF], sin[:, HALF:]], axis=-1).astype(np.float32)

    def to_c(x):
        return np.ascontiguousarray(
            x.reshape(nw, W, E).transpose(1, 0, 2).reshape(W, nw * E)
        ).astype(NP_F16)

    return to_c(cos), to_c(sinA)


def _mask01():
    j = np.arange(W)[:, None]
    i = np.arange(W)[None, :]
    m = (i >= j).astype(np.float32)
    return np.concatenate([m, m, m, m], axis=1).astype(NP_F16)  # [128, 512]


# ---------------------------------------------------------------- device body
def _body(ctx, tc, out_ap, q_ap, k_ap, v_ap, cos_ap, sinA_ap, mask_ap, ident_ap,
          n_bh, nw):
    nc = tc.nc
    FB = nw * E
    t_len = nw * W
    n_grp = nw // WT

    const = ctx.enter_context(tc.tile_pool(name="const", bufs=1))
    big = ctx.enter_context(tc.tile_pool(name="big", bufs=2))
    tbuf = ctx.enter_context(tc.tile_pool(name="tbuf", bufs=1))
    ering = ctx.enter_context(tc.tile_pool(name="ering", bufs=3))
    small = ctx.enter_context(tc.tile_pool(name="small", bufs=4))
    ptp = ctx.enter_context(tc.tile_pool(name="ptp", bufs=2, space="PSUM"))
    pst = ctx.enter_context(tc.tile_pool(name="pst", bufs=2, space="PSUM"))
    pov = ctx.enter_context(tc.tile_pool(name="pov", bufs=2, space="PSUM"))

    cos_c = const.tile([128, FB], F16)
    nc.sync.dma_start(cos_c[:, :], cos_ap)
    sinA_c = const.tile([128, FB], F16)
    nc.sync.dma_start(sinA_c[:, :], sinA_ap)
    mask_c = const.tile([128, 512], F16)
    nc.sync.dma_start(mask_c[:, :], mask_ap)
    ident_c = const.tile([128, 128], F16)
    nc.sync.dma_start(ident_c[:, :], ident_ap)

    for bh in range(n_bh):
        qn = big.tile([128, FB], F16)
        nc.gpsimd.dma_start(
            qn.rearrange("p (n e) -> p n e", e=E),
            q_ap[bh].rearrange("(n p) e -> p n e", p=128),
        )
        kn = big.tile([128, FB], F16)
        nc.gpsimd.dma_start(
            kn.rearrange("p (n e) -> p n e", e=E),
            k_ap[bh].rearrange("(n p) e -> p n e", p=128),
        )
        vx = big.tile([128, nw * (E + 1)], F16)
        vx3 = vx.rearrange("p (n c) -> p n c", c=E + 1)
        nc.gpsimd.dma_start(
            vx3[:, :, 0:E], v_ap[bh].rearrange("(n p) e -> p n e", p=128)
        )
        nc.gpsimd.memset(vx3[:, :, E : E + 1], 1.0)

        # rope: x' = x*cos + swapped(x)*sinA (all bf16, DVE 2x mode)
        def rope(xn, name):
            t2 = big.tile([128, FB], F16, name=f"t2_{name}")
            xb = big.tile([128, FB], F16, name=f"xb_{name}")
            x4 = xn.rearrange("p (n two h) -> p n two h", two=2, h=HALF)
            t4 = t2.rearrange("p (n two h) -> p n two h", two=2, h=HALF)
            s4 = sinA_c.rearrange("p (n two h) -> p n two h", two=2, h=HALF)
            nc.vector.tensor_mul(t4[:, :, 0, :], x4[:, :, 1, :], s4[:, :, 0, :])
            nc.vector.tensor_mul(t4[:, :, 1, :], x4[:, :, 0, :], s4[:, :, 1, :])
            xc = tbuf.tile([128, FB], F16, name=f"xc_{name}", tag="xc")
            nc.vector.tensor_mul(xc[:, :], xn[:, :], cos_c[:, :])
            nc.vector.tensor_add(xb[:, :], xc[:, :], t2[:, :])
            return xb

        qrb = rope(qn, "q")
        krb = rope(kn, "k")

        # transposes: one merged PSUM bank per 4 windows (q 0:512, k 512:1024)
        qT = tbuf.tile([64, t_len], F16)
        kT = tbuf.tile([64, t_len], F16)
        for g in range(n_grp):
            pt = ptp.tile([64, 2 * WT * 128], F16)
            for j in range(WT):
                w = g * WT + j
                nc.tensor.matmul(
                    pt[:, j * 128 : (j + 1) * 128],
                    qrb[:, w * E : (w + 1) * E], ident_c[:, :],
                    is_transpose=True,
                )
                nc.tensor.matmul(
                    pt[:, 512 + j * 128 : 512 + (j + 1) * 128],
                    krb[:, w * E : (w + 1) * E], ident_c[:, :],
                    is_transpose=True,
                )
            sl = slice(g * WT * 128, (g + 1) * WT * 128)
            nc.vector.tensor_copy(qT[:, sl], pt[:, 0:512])
            nc.scalar.copy(kT[:, sl], pt[:, 512:1024])

        # score groups of 4 windows / exp / mask / PV / normalize
        out_b = tbuf.tile([128, FB], F32, name="out_b", tag="out_b", bufs=2)
        e_tiles = [None] * n_grp
        for g in range(n_grp):
            st = pst.tile([128, 1024], F32)
            lim = 1024
            for j in range(WT):
                w = g * WT + j
                nc.tensor.matmul(
                    st[:, j * W : (j + 1) * W],
                    kT[:, w * W : (w + 1) * W],
                    qT[:, w * W : (w + 1) * W],
                )
                if w + 1 < nw:
                    nc.tensor.matmul(
                        st[:, 512 + j * W : 512 + (j + 1) * W],
                        kT[:, w * W : (w + 1) * W],
                        qT[:, (w + 1) * W : (w + 2) * W],
                    )
                else:
                    lim = 512 + j * W
            et = ering.tile([128, 1024], F16)
            nc.scalar.activation(
                et[:, 0:lim], st[:, 0:lim],
                mybir.ActivationFunctionType.Exp, scale=float(E) ** -0.5,
            )
            nc.vector.tensor_mul(et[:, 0:512], et[:, 0:512], mask_c[:, :])
            e_tiles[g] = et

            ov = pov.tile([128, WT * (E + 1)], F32, name="ov")
            for j in range(WT):
                w = g * WT + j
                osl = slice(j * (E + 1), (j + 1) * (E + 1))
                e_cur = et[:, j * W : (j + 1) * W]
                if w == 0:
                    nc.tensor.matmul(
                        ov[:, osl], e_cur, vx3[:, 0, :], start=True, stop=True
                    )
                else:
                    if j == 0:
                        e_prev = e_tiles[g - 1][:, 896:1024]
                    else:
                        e_prev = et[:, 512 + (j - 1) * W : 512 + j * W]
                    nc.tensor.matmul(
                        ov[:, osl], e_prev, vx3[:, w - 1, :], start=True, stop=False
                    )
                    nc.tensor.matmul(
                        ov[:, osl], e_cur, vx3[:, w, :], start=False, stop=True
                    )

            ov3 = ov.rearrange("p (n c) -> p n c", c=E + 1)
            r = small.tile([128, WT], F32)
            nc.vector.reciprocal(r[:, :], ov3[:, :, E])
            rb = r[:, :, None].broadcast_to([128, WT, E])
            osl2 = out_b.rearrange("p (n e) -> p n e", e=E)[
                :, g * WT : (g + 1) * WT, :
            ]
            nc.vector.tensor_mul(osl2, ov3[:, :, 0:E], rb)

        nc.sync.dma_start(
            out_ap[bh].rearrange("(n p) e -> p n e", p=128),
            out_b.rearrange("p (n e) -> p n e", e=E),
        )


# ---------------------------------------------------------------- build & run
_CACHE = {}


def _build():
    if "nc" in _CACHE:
        return _CACHE["nc"]
    nc = bacc.Bacc(
        "TRN2",
        target_bir_lowering=False,
        debug=False,
        enable_asserts=True,
        num_devices=N_CORES,
    )
    shp = [N_BH, T, E]
    q = nc.dram_tensor("q", shp, F32, kind="ExternalInput").ap()
    k = nc.dram_tensor("k", shp, F32, kind="ExternalInput").ap()
    v = nc.dram_tensor("v", shp, F32, kind="ExternalInput").ap()
    cos = nc.dram_tensor("cos", [128, NW * E], F16, kind="ExternalInput").ap()
    sinA = nc.dram_tensor("sinA", [128, NW * E], F16, kind="ExternalInput").ap()
    mask = nc.dram_tensor("mask", [128, 512], F16, kind="ExternalInput").ap()
    ident = nc.dram_tensor("ident", [128, 128], F16, kind="ExternalInput").ap()
    out = nc.dram_tensor("out", shp, F32, kind="ExternalOutput").ap()
    with tile.TileContext(nc) as tc:
        with ExitStack() as ctx:
            _body(ctx, tc, out, q, k, v, cos, sinA, mask, ident, N_BH, NW)
    nc.finalize()
    _CACHE["nc"] = nc
    return nc


def kernel(q, k, v, trace=False):
    nc = _build()
    cos_c, sinA_c = _rope_tables()
    mask = _mask01()
    ident = np.eye(128, dtype=np.float32).astype(NP_F16)

    qm = np.ascontiguousarray(q.reshape(B * H, T, E))
    km = np.ascontiguousarray(k.reshape(B * H, T, E))
    vm = np.ascontiguousarray(v.reshape(B * H, T, E))
    in_maps = []
    for c in range(N_CORES):
        s = slice(c * N_BH, (c + 1) * N_BH)
        in_maps.append(
            {
                "q": np.ascontiguousarray(qm[s]),
                "k": np.ascontiguousarray(km[s]),
                "v": np.ascontiguousarray(vm[s]),
                "cos": cos_c,
                "sinA": sinA_c,
                "mask": mask,
                "ident": ident,
            }
        )
    res = bass_utils.run_bass_kernel_spmd(
        nc, in_maps, core_ids=list(range(N_CORES)), trace=trace
    )
    out = np.concatenate([r["out"] for r in res.results], axis=0)
    out = out.reshape(q.shape).astype(np.float32)
    if trace:
        return out, res
    return out


# revision 15
# speedup vs baseline: 1.0555x; 1.0555x over previous
"""Local (windowed causal) attention with RoPE — Trainium2 Bass kernel.

Problem: B=4, H=16, T=4096, E=64, WINDOW=128, look_backward=1, causal.
Sharding: merged batch*heads (64 rows) split 8 per NeuronCore across 8 cores.

v3 design (all matmul operands fp16, fp32 accumulation):
- q/k/v loaded per bh via SWDGE (gpsimd) with f32->bf16 cast, "(c)-layout"
  [128, nw*64] (tile[p, w*64+e] = x[w*128+p, e]).
- RoPE in bf16 on DVE (whole-bh ops, 2x mode), host-precomputed bf16 tables.
- PE transposes into one merged PSUM bank [64, 1024] per 4 windows
  (q cols 0:512, k cols 512:1024) -> qT/kT [64, T] bf16; copies on ACT.
- Scores: 4 windows per PSUM tile [128, 1024] f32; st[j,i] via
  matmul(lhsT=kT_w [64,128], rhs=qT[w..w+2) [64,256]); one ACT exp
  (scale=8^-0.5... E**-0.5) -> E bf16 [128,1024]; DVE causal mask on cur
  halves (strided, mul by 0/1 bf16).
- PV: matmul(lhsT=E block [128,128] bf16, rhs=[v|1] bf16 [128,65]) accumulating
  out + softmax denominator in PSUM [128, 260] (4 windows); batched DVE
  reciprocal + broadcast-mul normalize into a per-bh out buffer; one 1MB
  store per bh on the sync ring.
"""

from contextlib import ExitStack

import numpy as np

import concourse.bass as bass
import concourse.bacc as bacc
import concourse.mybir as mybir
from concourse import tile
from concourse import bass_utils

F32 = mybir.dt.float32
F16 = mybir.dt.float16
NP_F16 = np.float16
E = 64
W = 128
HALF = 32
B, H, T = 4, 16, 4096
N_CORES = 8
N_BH = (B * H) // N_CORES
NW = T // W
WT = 4  # windows per transpose/score/output batch


# ---------------------------------------------------------------- host consts
def _rope_tables(t_len=T):
    nw = t_len // W
    inv_freq = 1.0 / (10000.0 ** (np.arange(0, E, 2, dtype=np.float32) / E))
    t = np.arange(t_len, dtype=np.float32)
    freqs = np.outer(t, inv_freq)
    emb = np.concatenate([freqs, freqs], axis=-1)
    cos = np.cos(emb).astype(np.float32)
    sin = np.sin(emb).astype(np.float32)
    sinA = np.concatenate([-sin[:, :HALF], sin[:, HALF:]], axis=-1).astype(np.float32)

    def to_c(x):
        return np.ascontiguousarray(
            x.reshape(nw, W, E).transpose(1, 0, 2).reshape(W, nw * E)
        ).astype(NP_F16)

    return to_c(cos), to_c(sinA)


def _mask01():
    j = np.arange(W)[:, None]
    i = np.arange(W)[None, :]
    m = (i >= j).astype(np.float32)
    return np.concatenate([m, m, m, m], axis=1).astype(NP_F16)  # [128, 512]


# ---------------------------------------------------------------- device body
def _body(ctx, tc, out_ap, q_ap, k_ap, v_ap, cos_ap, sinA_ap, mask_ap, ident_ap,
          n_bh, nw):
    nc = tc.nc
    FB = nw * E
    t_len = nw * W
    n_grp = nw // WT

    const = ctx.enter_context(tc.tile_pool(name="const", bufs=1))
    big = ctx.enter_context(tc.tile_pool(name="big", bufs=2))
    tbuf = ctx.enter_context(tc.tile_pool(name="tbuf", bufs=1))
    ering = ctx.enter_context(tc.tile_pool(name="ering", bufs=4))
    small = ctx.enter_context(tc.tile_pool(name="small", bufs=4))
    ptp = ctx.enter_context(tc.tile_pool(name="ptp", bufs=2, space="PSUM"))
    pst = ctx.enter_context(tc.tile_pool(name="pst", bufs=2, space="PSUM"))
    pov = ctx.enter_context(tc.tile_pool(name="pov", bufs=2, space="PSUM"))

    cos_c = const.tile([128, FB], F16)
    nc.sync.dma_start(cos_c[:, :], cos_ap)
    sinA_c = const.tile([128, FB], F16)
    nc.sync.dma_start(sinA_c[:, :], sinA_ap)
    mask_c = const.tile([128, 512], F16)
    nc.sync.dma_start(mask_c[:, :], mask_ap)
    ident_c = const.tile([128, 128], F16)
    nc.sync.dma_start(ident_c[:, :], ident_ap)

    for bh in range(n_bh):
        qn = big.tile([128, FB], F16)
        nc.gpsimd.dma_start(
            qn.rearrange("p (n e) -> p n e", e=E),
            q_ap[bh].rearrange("(n p) e -> p n e", p=128),
        )
        kn = big.tile([128, FB], F16)
        nc.gpsimd.dma_start(
            kn.rearrange("p (n e) -> p n e", e=E),
            k_ap[bh].rearrange("(n p) e -> p n e", p=128),
        )
        vx = big.tile([128, nw * (E + 1)], F16)
        vx3 = vx.rearrange("p (n c) -> p n c", c=E + 1)
        nc.gpsimd.dma_start(
            vx3[:, :, 0:E], v_ap[bh].rearrange("(n p) e -> p n e", p=128)
        )
        nc.gpsimd.memset(vx3[:, :, E : E + 1], 1.0)

        # rope: x' = x*cos + swapped(x)*sinA (all bf16, DVE 2x mode)
        def rope(xn, name):
            t2 = big.tile([128, FB], F16, name=f"t2_{name}")
            xb = big.tile([128, FB], F16, name=f"xb_{name}")
            x4 = xn.rearrange("p (n two h) -> p n two h", two=2, h=HALF)
            t4 = t2.rearrange("p (n two h) -> p n two h", two=2, h=HALF)
            s4 = sinA_c.rearrange("p (n two h) -> p n two h", two=2, h=HALF)
            nc.vector.tensor_mul(t4[:, :, 0, :], x4[:, :, 1, :], s4[:, :, 0, :])
            nc.vector.tensor_mul(t4[:, :, 1, :], x4[:, :, 0, :], s4[:, :, 1, :])
            xc = tbuf.tile([128, FB], F16, name=f"xc_{name}", tag="xc")
            nc.vector.tensor_mul(xc[:, :], xn[:, :], cos_c[:, :])
            nc.vector.tensor_add(xb[:, :], xc[:, :], t2[:, :])
            return xb

        qrb = rope(qn, "q")
        krb = rope(kn, "k")

        # transposes: one merged PSUM bank per 4 windows (q 0:512, k 512:1024)
        qT = tbuf.tile([64, t_len], F16)
        kT = tbuf.tile([64, t_len], F16)
        for g in range(n_grp):
            pt = ptp.tile([64, 2 * WT * 128], F16)
            for j in range(WT):
                w = g * WT + j
                nc.tensor.matmul(
                    pt[:, j * 128 : (j + 1) * 128],
                    qrb[:, w * E : (w + 1) * E], ident_c[:, :],
                    is_transpose=True,
                )
                nc.tensor.matmul(
                    pt[:, 512 + j * 128 : 512 + (j + 1) * 128],
                    krb[:, w * E : (w + 1) * E], ident_c[:, :],
                    is_transpose=True,
                )
            sl = slice(g * WT * 128, (g + 1) * WT * 128)
            nc.vector.tensor_copy(qT[:, sl], pt[:, 0:512])
            nc.scalar.copy(kT[:, sl], pt[:, 512:1024])

        # score groups of 4 windows, software-pipelined:
        # emit QK/exp/mask for group g, PV + normalize for group g-1
        out_b = tbuf.tile([128, FB], F32, name="out_b", tag="out_b", bufs=2)
        e_tiles = [None] * n_grp

        def emit_scores(g):
            st = pst.tile([128, 1024], F32, name="st")
            lim = 1024
            for j in range(WT):
                w = g * WT + j
                nc.tensor.matmul(
                    st[:, j * W : (j + 1) * W],
                    kT[:, w * W : (w + 1) * W],
                    qT[:, w * W : (w + 1) * W],
                )
                if w + 1 < nw:
                    nc.tensor.matmul(
                        st[:, 512 + j * W : 512 + (j + 1) * W],
                        kT[:, w * W : (w + 1) * W],
                        qT[:, (w + 1) * W : (w + 2) * W],
                    )
                else:
                    lim = 512 + j * W
            et = ering.tile([128, 1024], F16, name="et")
            nc.scalar.activation(
                et[:, 0:lim], st[:, 0:lim],
                mybir.ActivationFunctionType.Exp, scale=float(E) ** -0.5,
            )
            nc.vector.tensor_mul(et[:, 0:512], et[:, 0:512], mask_c[:, :])
            e_tiles[g] = et

        def emit_pv(g):
            et = e_tiles[g]
            ov = pov.tile([128, WT * (E + 1)], F32, name="ov")
            for j in range(WT):
                w = g * WT + j
                osl = slice(j * (E + 1), (j + 1) * (E + 1))
                e_cur = et[:, j * W : (j + 1) * W]
                if w == 0:
                    nc.tensor.matmul(
                        ov[:, osl], e_cur, vx3[:, 0, :], start=True, stop=True
                    )
                else:
                    if j == 0:
                        e_prev = e_tiles[g - 1][:, 896:1024]
                    else:
                        e_prev = et[:, 512 + (j - 1) * W : 512 + j * W]
                    nc.tensor.matmul(
                        ov[:, osl], e_prev, vx3[:, w - 1, :], start=True, stop=False
                    )
                    nc.tensor.matmul(
                        ov[:, osl], e_cur, vx3[:, w, :], start=False, stop=True
                    )
            ov3 = ov.rearrange("p (n c) -> p n c", c=E + 1)
            r = small.tile([128, WT], F32, name="r")
            nc.vector.reciprocal(r[:, :], ov3[:, :, E])
            rb = r[:, :, None].broadcast_to([128, WT, E])
            osl2 = out_b.rearrange("p (n e) -> p n e", e=E)[
                :, g * WT : (g + 1) * WT, :
            ]
            nc.vector.tensor_mul(osl2, ov3[:, :, 0:E], rb)

        for g in range(n_grp + 1):
            if g < n_grp:
                emit_scores(g)
            if g >= 1:
                emit_pv(g - 1)

        nc.sync.dma_start(
            out_ap[bh].rearrange("(n p) e -> p n e", p=128),
            out_b.rearrange("p (n e) -> p n e", e=E),
        )


# ---------------------------------------------------------------- build & run
_CACHE = {}


def _build():
    if "nc" in _CACHE:
        return _CACHE["nc"]
    nc = bacc.Bacc(
        "TRN2",
        target_bir_lowering=False,
        debug=False,
        enable_asserts=True,
        num_devices=N_CORES,
    )
    shp = [N_BH, T, E]
    q = nc.dram_tensor("q", shp, F32, kind="ExternalInput").ap()
    k = nc.dram_tensor("k", shp, F32, kind="ExternalInput").ap()
    v = nc.dram_tensor("v", shp, F32, kind="ExternalInput").ap()
    cos = nc.dram_tensor("cos", [128, NW * E], F16, kind="ExternalInput").ap()
    sinA = nc.dram_tensor("sinA", [128, NW * E], F16, kind="ExternalInput").ap()
    mask = nc.dram_tensor("mask", [128, 512], F16, kind="ExternalInput").ap()
    ident = nc.dram_tensor("ident", [128, 128], F16, kind="ExternalInput").ap()
    out = nc.dram_tensor("out", shp, F32, kind="ExternalOutput").ap()
    with tile.TileContext(nc) as tc:
        with ExitStack() as ctx:
            _body(ctx, tc, out, q, k, v, cos, sinA, mask, ident, N_BH, NW)
    nc.finalize()
    _CACHE["nc"] = nc
    return nc


def kernel(q, k, v, trace=False):
    nc = _build()
    cos_c, sinA_c = _rope_tables()
    mask = _mask01()
    ident = np.eye(128, dtype=np.float32).astype(NP_F16)

    qm = np.ascontiguousarray(q.reshape(B * H, T, E))
    km = np.ascontiguousarray(k.reshape(B * H, T, E))
    vm = np.ascontiguousarray(v.reshape(B * H, T, E))
    in_maps = []
    for c in range(N_CORES):
        s = slice(c * N_BH, (c + 1) * N_BH)
        in_maps.append(
            {
                "q": np.ascontiguousarray(qm[s]),
                "k": np.ascontiguousarray(km[s]),
                "v": np.ascontiguousarray(vm[s]),
                "cos": cos_c,
                "sinA": sinA_c,
                "mask": mask,
                "ident": ident,
            }
        )
    res = bass_utils.run_bass_kernel_spmd(
        nc, in_maps, core_ids=list(range(N_CORES)), trace=trace
    )
    out = np.concatenate([r["out"] for r in res.results], axis=0)
    out = out.reshape(q.shape).astype(np.float32)
    if trace:
        return out, res
    return out
